# revision 13
# baseline (speedup 1.0000x reference)
"""Bahdanau additive attention Trainium2 Bass kernel.

Reference (per batch b):
    U = key @ W_encoder.T                  # [S, A]
    V = q @ W_decoder.T                    # [A]
    score = tanh(U + V) @ v[0]             # [S]
    w = softmax(score)                     # [S]
    context = w @ key                      # [KD]

Sharding: data-parallel over batch across 8 NeuronCores (4 batches/core),
weights replicated.  All heavy matmuls run in fp32r (fp32 with 11-bit
mantissa, full PE rate at free-dim >= 256, ~2e-4 relative error).

Per-core pipeline, per batch, per s-chunk of 512:
  1. SWDGE DMA-cast key chunk fp32->fp32r into SBUF (native [s,k] layout).
  2. keyT [k, s] tiles via normal-mode PE matmuls out = key_block.T @ I
     (contraction over the s partition dim; measured much faster than the
     dedicated transpose mode for this 128x128 fp32r case).
  3. U^T tiles [a=128, s=512] = WeT.T @ keyT accumulated over k in PSUM.
  4. ACT: tanh(U^T + V[a]) from PSUM (V as per-partition bias), fp32r out.
  5. score [1, 512] = v.T @ tanh-tiles accumulated over a-tiles on PE.
  6. ACT: e = exp(score) (no max subtraction needed: |score| <= sum|v| ~ 26,
     well inside fp32 range), accum_out gives the chunk's sum(e).
  7. PE-transpose e-row into an e-column tile [128, 4].
  8. context PSUM [1, 1024] += e-col.T @ key-native, accumulated across all
     chunks; key chunk is then dead (single pass over key).
Batch epilogue: Z = sum of chunk sums, context * (1/Z) on DVE, DMA out.
"""
import sys
sys.path.insert(0, "/opt/trn_rl_repo")

from contextlib import ExitStack

import numpy as np

import concourse.bass as bass
import concourse.tile as tile
from concourse import bacc, masks, mybir

dt = mybir.dt
AF = mybir.ActivationFunctionType

# Full problem shape
B, S, KD, QD, AD = 32, 2048, 1024, 1024, 1024
N_CORES = 8
BS = B // N_CORES          # batches per core
SC = 512                   # s-chunk (columns per U matmul)


def build_kernel(nc, bs=BS, s=S, kd=KD, qd=QD, ad=AD, reps=1, dyn_reps=0):
    """Emit the per-core kernel into `nc` (a bacc.Bacc).

    reps>1 statically unrolls the whole pipeline (timing amplification);
    dyn_reps>0 instead wraps it in a hardware For_i loop.
    """
    f32, f32r = dt.float32, dt.float32r
    nsc = s // SC            # s-chunks per batch
    nkt = kd // 128          # k-tiles
    nat = ad // 128          # a-tiles
    nqt = qd // 128          # q-tiles
    assert s % SC == 0 and kd % 128 == 0 and ad % 128 == 0 and qd % 128 == 0

    key_d = nc.dram_tensor("key", [bs, s, kd], f32, kind="ExternalInput").ap()
    q_d = nc.dram_tensor("q", [bs, qd], f32, kind="ExternalInput").ap()
    we_d = nc.dram_tensor("W_encoder", [ad, kd], f32, kind="ExternalInput").ap()
    wd_d = nc.dram_tensor("W_decoder", [ad, qd], f32, kind="ExternalInput").ap()
    v_d = nc.dram_tensor("v", [1, ad], f32, kind="ExternalInput").ap()
    out_d = nc.dram_tensor("out", [bs, kd], f32, kind="ExternalOutput").ap()

    with tile.TileContext(nc) as tc, ExitStack() as ctx:
        const = ctx.enter_context(tc.tile_pool(name="const", bufs=1))

        ident_f = const.tile([128, 128], f32, name="ident_f")
        masks.make_identity(nc, ident_f[:])
        ident_r = const.tile([128, 128], f32r, name="ident_r")
        nc.vector.tensor_copy(ident_r[:], ident_f[:])
        one_f = const.tile([1, 1], f32, name="one_f")
        nc.gpsimd.memset(one_f[:], 1.0)
        one_r = const.tile([1, 1], f32r, name="one_r")
        nc.vector.tensor_copy(one_r[:], one_f[:])

        # ---------------- weight prep (once per core) ----------------
        # WeT[k, a] tiles (fp32r), one [128, ad] tile per k-tile.
        wet = [const.tile([128, ad], f32r, name=f"wet{t}") for t in range(nkt)]
        # V bias [a-tile][128, bs] fp32 and v columns [128, nat] fp32r.
        vbias = [const.tile([128, bs], f32, name=f"vbias{m}") for m in range(nat)]
        vcols = const.tile([128, nat], f32r, name="vcols")

        with (
            tc.tile_pool(name="wprep", bufs=1) as wprep,
            tc.tile_pool(name="wpsum", bufs=2, space="PSUM") as wpsum,
        ):
            # --- WeT --- (weights DMA-cast to fp32r so the transpose
            # matmuls run at the fp32r rate)
            we_nat = [wprep.tile([128, kd], f32r, name=f"wenat{m}", tag=f"wn{m}")
                      for m in range(nat)]
            for m in range(nat):
                nc.gpsimd.dma_start(we_nat[m][:], we_d[m * 128:(m + 1) * 128, :])
            for t in range(nkt):
                for half in range(0, nat, 4):
                    n = min(4, nat - half)
                    ps = wpsum.tile([128, 512], f32, name=f"wps{t}_{half}",
                                    tag="wps")
                    for j in range(n):
                        nc.tensor.matmul(
                            ps[:, j * 128:(j + 1) * 128],
                            we_nat[half + j][:, t * 128:(t + 1) * 128],
                            ident_r[:], start=True, stop=True)
                    nc.vector.tensor_copy(
                        wet[t][:, half * 128:(half + n) * 128],
                        ps[:, :n * 128])

            # --- WdT (transient) + qT + V matmul ---
            wd_nat = we_nat  # reuse the same sbuf tiles (tags) for Wd
            for m in range(nat):
                nc.gpsimd.dma_start(wd_nat[m][:], wd_d[m * 128:(m + 1) * 128, :])
            wdt = [wprep.tile([128, ad], f32r, name=f"wdt{t}", tag=f"wdt{t}")
                   for t in range(nqt)]
            for t in range(nqt):
                for half in range(0, nat, 4):
                    n = min(4, nat - half)
                    ps = wpsum.tile([128, 512], f32, name=f"wdps{t}_{half}",
                                    tag="wps")
                    for j in range(n):
                        nc.tensor.matmul(
                            ps[:, j * 128:(j + 1) * 128],
                            wd_nat[half + j][:, t * 128:(t + 1) * 128],
                            ident_r[:], start=True, stop=True)
                    nc.vector.tensor_copy(
                        wdt[t][:, half * 128:(half + n) * 128],
                        ps[:, :n * 128])

            qn = wprep.tile([bs, qd], f32, name="qn")
            nc.sync.dma_start(qn[:], q_d)
            psq = wpsum.tile([128, nqt * bs], f32, name="psq", tag="psq")
            for t in range(nqt):
                nc.tensor.matmul(psq[:, t * bs:(t + 1) * bs],
                                 qn[:, t * 128:(t + 1) * 128],
                                 ident_f[:bs, :bs], is_transpose=True)
            qt = wprep.tile([128, nqt * bs], f32r, name="qt")
            nc.vector.tensor_copy(qt[:], psq[:])

            for m in range(nat):
                psv = wpsum.tile([128, bs], f32, name=f"psv{m}", tag="psv")
                for t in range(nqt):
                    nc.tensor.matmul(psv[:], wdt[t][:, m * 128:(m + 1) * 128],
                                     qt[:, t * bs:(t + 1) * bs],
                                     start=(t == 0), stop=(t == nqt - 1))
                nc.vector.tensor_copy(vbias[m][:], psv[:])

            # --- v columns ---
            vrow = wprep.tile([1, ad], f32, name="vrow")
            nc.sync.dma_start(vrow[:], v_d)
            psvc = wpsum.tile([128, nat], f32, name="psvc", tag="psv")
            for m in range(nat):
                nc.tensor.matmul(psvc[:, m:m + 1],
                                 vrow[:, m * 128:(m + 1) * 128],
                                 one_f[:], is_transpose=True)
            nc.vector.tensor_copy(vcols[:], psvc[:])

        # ---------------- main streaming loop ----------------
        kpool = ctx.enter_context(tc.tile_pool(name="knat", bufs=3))
        ktpool = ctx.enter_context(tc.tile_pool(name="keyT", bufs=2))
        thpool = ctx.enter_context(tc.tile_pool(name="tanh", bufs=2))
        spool = ctx.enter_context(tc.tile_pool(name="small", bufs=2))
        pp_t = ctx.enter_context(tc.tile_pool(name="pp_t", bufs=2, space="PSUM"))
        pp_u = ctx.enter_context(tc.tile_pool(name="pp_u", bufs=2, space="PSUM"))
        pp_s = ctx.enter_context(tc.tile_pool(name="pp_s", bufs=1, space="PSUM"))
        pp_c = ctx.enter_context(tc.tile_pool(name="pp_c", bufs=1, space="PSUM"))

        nkh = kd // 512  # context free-dim chunks

        def emit_body(rep):
            chunks = [(b, c) for b in range(bs) for c in range(nsc)]
            G = len(chunks)
            knat3s = {}
            kts_map = {}
            bstate = {}

            def emit_dma(g):
                b, c = chunks[g]
                knat = kpool.tile([128, 4 * kd], f32r,
                                  name=f"knatr{rep}g{g}", tag="knat")
                knat3 = knat[:].rearrange("p (t k) -> p t k", k=kd)
                nc.gpsimd.dma_start(
                    knat3,
                    key_d[b, c * SC:(c + 1) * SC, :]
                    .rearrange("(t p) k -> p t k", p=128))
                knat3s[g] = knat3

            def tp_ops(g):
                # keyT tiles via normal-mode PE "transpose":
                # out = knat_block.T @ I (contraction over the s partition
                # dim) — much faster than transpose-mode for 128x128 fp32r.
                # Returns a list of emitters so the transposes of chunk g
                # can interleave between the U matmuls of chunk g-1 (their
                # weight loads hide under the N=512 U streams).
                knat3 = knat3s[g]
                kts = []
                kts_map[g] = kts
                ops = []
                for t in range(nkt):
                    pst = pp_t.tile([128, SC], f32,
                                    name=f"pstr{rep}g{g}t{t}", tag="pst")
                    for sp in range(4):
                        def op(pst=pst, sp=sp, t=t, knat3=knat3):
                            nc.tensor.matmul(
                                pst[:, sp * 128:(sp + 1) * 128],
                                knat3[:, sp, t * 128:(t + 1) * 128],
                                ident_r[:], start=True, stop=True)
                        ops.append(op)
                    kt = ktpool.tile([128, SC], f32r,
                                     name=f"ktr{rep}g{g}t{t}", tag=f"kt{t}")
                    kts.append(kt)

                    def evac(kt=kt, pst=pst):
                        nc.vector.tensor_copy(kt[:], pst[:])
                    ops.append(evac)
                return ops

            def emit_compute(g, il):
                b, c = chunks[g]
                tagb = f"r{rep}b{b}"
                knat3 = knat3s[g]
                kts = kts_map[g]
                if c == 0:
                    zparts = spool.tile([1, nsc], f32, name=f"zp{tagb}",
                                        tag="zparts")
                    ctx_ps = [pp_c.tile([1, 512], f32, name=f"ctx{tagb}_{h}",
                                        tag=f"ctx{h}") for h in range(nkh)]
                    bstate[b] = (zparts, ctx_ps)
                zparts, ctx_ps = bstate[b]

                # 3+4. U^T a-tiles (with next chunk's transposes woven in),
                # tanh(U+V) on ACT
                ili = 0
                ths = []
                for m in range(nat):
                    psu = pp_u.tile([128, SC], f32,
                                    name=f"psu{tagb}c{c}m{m}", tag="psu")
                    for t in range(nkt):
                        nc.tensor.matmul(
                            psu[:], wet[t][:, m * 128:(m + 1) * 128],
                            kts[t][:],
                            start=(t == 0), stop=(t == nkt - 1))
                        if t % 2 == 1 and ili < len(il):
                            il[ili]()
                            ili += 1
                    th = thpool.tile([128, SC], f32r,
                                     name=f"th{tagb}c{c}m{m}", tag=f"th{m}")
                    nc.scalar.activation(th[:], psu[:], AF.Tanh,
                                         bias=vbias[m][:, b:b + 1])
                    ths.append(th)
                while ili < len(il):
                    il[ili]()
                    ili += 1

                # 5. score row
                pss = pp_s.tile([1, SC], f32, name=f"pss{tagb}c{c}",
                                tag="pss")
                for m in range(nat):
                    nc.tensor.matmul(pss[:], vcols[:, m:m + 1], ths[m][:],
                                     start=(m == 0), stop=(m == nat - 1))

                # 6. e = exp(score); chunk sum via accum_out
                erow = spool.tile([1, SC], f32, name=f"erow{tagb}c{c}",
                                  tag="erow")
                nc.scalar.activation(erow[:], pss[:], AF.Exp,
                                     accum_out=zparts[:, c:c + 1])

                # 7. e-row -> e-columns [128, 4] (fp32 transpose-mode;
                # a normal fp32 K=1 matmul here measured ~2us each)
                pse = pp_s.tile([128, 4], f32, name=f"pse{tagb}c{c}",
                                tag="pse")
                for sp in range(4):
                    nc.tensor.matmul(pse[:, sp:sp + 1],
                                     erow[:, sp * 128:(sp + 1) * 128],
                                     one_f[:], is_transpose=True)
                ecol = spool.tile([128, 4], f32r, name=f"ec{tagb}c{c}",
                                  tag="ecol")
                nc.vector.tensor_copy(ecol[:], pse[:])

                # 8. context accumulation (contract over s)
                for sp in range(4):
                    for h in range(nkh):
                        nc.tensor.matmul(
                            ctx_ps[h][:], ecol[:, sp:sp + 1],
                            knat3[:, sp, h * 512:(h + 1) * 512],
                            start=(c == 0 and sp == 0),
                            stop=(c == nsc - 1 and sp == 3))

                if c == nsc - 1:
                    # batch epilogue: normalize and store
                    z = spool.tile([1, 1], f32, name=f"z{tagb}", tag="z")
                    nc.vector.reduce_sum(z[:], zparts[:],
                                         axis=mybir.AxisListType.X)
                    rz = spool.tile([1, 1], f32, name=f"rz{tagb}", tag="rz")
                    nc.vector.reciprocal(rz[:], z[:])
                    cout = spool.tile([1, kd], f32, name=f"cout{tagb}",
                                      tag="cout")
                    for h in range(nkh):
                        nc.vector.tensor_scalar_mul(
                            cout[:, h * 512:(h + 1) * 512],
                            ctx_ps[h][:], rz[:])
                    nc.sync.dma_start(out_d[b:b + 1, :], cout[:])

            emit_dma(0)
            first_ops = tp_ops(0)
            for op in first_ops:
                op()
            for g in range(G):
                if g + 1 < G:
                    emit_dma(g + 1)
                    il = tp_ops(g + 1)
                else:
                    il = []
                emit_compute(g, il)

        if dyn_reps:
            with tc.For_i(0, dyn_reps, 1):
                emit_body(0)
        else:
            for rep in range(reps):
                emit_body(rep)
    return nc


_CACHE = {}


def _get_compiled(cfg):
    if cfg not in _CACHE:
        nc = bacc.Bacc("TRN2", target_bir_lowering=False, debug=False)
        build_kernel(nc, *cfg)
        nc.compile()
        _CACHE[cfg] = nc
    return _CACHE[cfg]


def kernel(**inputs):
    from concourse.bass_utils import run_bass_kernel_spmd

    key = np.asarray(inputs["key"], dtype=np.float32)
    q = np.asarray(inputs["q"], dtype=np.float32)
    we = np.asarray(inputs["W_encoder"], dtype=np.float32)
    wd = np.asarray(inputs["W_decoder"], dtype=np.float32)
    v = np.asarray(inputs["v"], dtype=np.float32)

    nc = _get_compiled((BS, S, KD, QD, AD, 1))
    in_maps = []
    for cidx in range(N_CORES):
        sl = slice(cidx * BS, (cidx + 1) * BS)
        in_maps.append({
            "key": key[sl], "q": q[sl],
            "W_encoder": we, "W_decoder": wd, "v": v,
        })
    res = run_bass_kernel_spmd(nc, in_maps, list(range(N_CORES))).results
    return np.concatenate([r["out"] for r in res], axis=0)


if __name__ == "__main__":
    # quick smoke: random small check against numpy on this module's math
    pass
